# revision 5
# baseline (speedup 1.0000x reference)
"""AttnBlock++ (GroupNorm -> q/k/v 1x1 -> full LxL attention -> proj -> residual)
on 8 Trainium2 NeuronCores, data-parallel over batch (one batch element per core).

Per-core dataflow (C=256 channels, L=2048 positions):
  - x (C,L) lives as two 128-partition tiles (c-tiles).
  - GroupNorm stats: per-channel bn_stats over L, then an 8-channel group
    combine via a tiny (128,16) selector matmul; broadcast back via its
    transpose; apply as per-partition scale/bias on ScalarE.
  - q,k: (C,L) layout (channel-major).  v: transposed (L,C) layout so the
    PV matmul needs no transposes of the probability matrix.
  - Attention over 4 query chunks of 512:
      scores^T block (key-major): sT[i,l] = sum_c k[c,i] q[c,l]  (PE)
      probs = exp(sT/16) elementwise (ScalarE, no max subtraction needed:
        scores are in [-11, 12], exp stays in fp32 range)
      denominator AND its across-partition broadcast in one shot:
        R[m,l] = sum_i 1 * probs[i,l] via an all-ones (128,128) lhsT (PE)
      attn[c,l] = (sum_i v^T[i,c] probs[i,l]) * (1/R) -- the divide is fused
        into the PSUM->SBUF copy on VectorE.
  - Output proj + residual; biases are folded in as K=1 rank-1 matmul
    accumulations or per-partition tensor_scalar adds.
  - All big matmuls run in float32r (~1.6e-4 max rel err, 4x faster than fp32).
"""

import numpy as np

import concourse.bacc as bacc
import concourse.mybir as mybir
import concourse.tile as tile
from concourse.bass_utils import run_bass_kernel_spmd

f32 = mybir.dt.float32
f32r = mybir.dt.float32r

B, C, L = 8, 256, 2048
G = 32
EPS = 1e-6
CT = C // 128            # 2 channel tiles
NCH = L // 512           # 4 query chunks
KB = L // 128            # 16 key blocks
SCALE = C ** -0.5        # 1/16

AF = mybir.ActivationFunctionType


def _build():
    nc = bacc.Bacc(trn_type="TRN2")

    x_d = nc.dram_tensor("x", (C, L), f32, kind="ExternalInput")
    w_d = [nc.dram_tensor(f"w{i}", (C, C), f32, kind="ExternalInput") for i in range(4)]
    b_d = [nc.dram_tensor(f"b{i}", (C,), f32, kind="ExternalInput") for i in range(4)]
    gam_d = nc.dram_tensor("gn_gamma", (C,), f32, kind="ExternalInput")
    bet_d = nc.dram_tensor("gn_beta", (C,), f32, kind="ExternalInput")
    out_d = nc.dram_tensor("out", (C, L), f32, kind="ExternalOutput")

    # group-combine selector (1/8 at [c, c//8]) and its broadcast transpose
    g8s_np = np.zeros((128, G // CT), np.float32)
    g8s_np[np.arange(128), np.arange(128) // 8] = 1.0 / 8.0
    g8t_np = np.zeros((G // CT, 128), np.float32)
    g8t_np[np.arange(128) // 8, np.arange(128)] = 1.0
    g8s_d = nc.inline_tensor(g8s_np, "g8s")
    g8t_d = nc.inline_tensor(g8t_np, "g8t")

    with tile.TileContext(nc) as tc:
        with tc.tile_pool(name="const", bufs=1) as cp, \
             tc.tile_pool(name="data", bufs=1) as dp, \
             tc.tile_pool(name="wstage", bufs=2) as wsp, \
             tc.tile_pool(name="small", bufs=1) as sp, \
             tc.tile_pool(name="expst", bufs=3) as ep, \
             tc.tile_pool(name="attn", bufs=2) as ap_, \
             tc.tile_pool(name="fin", bufs=4) as fp_, \
             tc.tile_pool(name="ps", bufs=1, space="PSUM") as ps:

            # ---------- persistent data tiles ----------
            xt = dp.tile([128, CT, L], f32, tag="xt", name="xt")
            hn = dp.tile([128, CT, L], f32r, tag="hn", name="hn")
            qt = dp.tile([128, CT, L], f32r, tag="qt", name="qt")
            kt = dp.tile([128, CT, L], f32r, tag="kt", name="kt")
            vT = dp.tile([128, KB, C], f32r, tag="vT", name="vT")

            # ---------- input DMAs ----------
            for t in range(CT):
                for j in range(4):
                    nc.sync.dma_start(
                        out=xt[:, t, j * 512:(j + 1) * 512],
                        in_=x_d[t * 128:(t + 1) * 128, j * 512:(j + 1) * 512])

            # weights: DMA fp32 stage then round to f32r
            wr = [cp.tile([128, CT, C], f32r, tag=f"w{i}r", name=f"w{i}r") for i in range(4)]
            for i in range(4):
                for t in range(CT):
                    stg = wsp.tile([128, C], f32, tag="wstage", name="wstage")
                    nc.sync.dma_start(out=stg[:], in_=w_d[i][t * 128:(t + 1) * 128, :])
                    nc.vector.tensor_copy(wr[i][:, t, :], stg[:])

            g8s_sb = cp.tile([128, G // CT], f32, tag="g8s", name="g8s")
            nc.sync.dma_start(out=g8s_sb[:], in_=g8s_d[:, :])
            g8t_sb = cp.tile([G // CT, 128], f32, tag="g8t", name="g8t")
            nc.sync.dma_start(out=g8t_sb[:], in_=g8t_d[:, :])

            ones_f = cp.tile([128, 128], f32, tag="onesf", name="onesf")
            nc.vector.memset(ones_f[:], 1.0)
            ones_r = cp.tile([128, 128], f32r, tag="ones", name="ones")
            nc.vector.tensor_copy(ones_r[:], ones_f[:])
            ones512_f = cp.tile([1, 512], f32, tag="ones512f", name="ones512f")
            nc.vector.memset(ones512_f[:], 1.0)
            ones512_r = cp.tile([1, 512], f32r, tag="ones512", name="ones512")
            nc.vector.tensor_copy(ones512_r[:], ones512_f[:])

            # per-channel-tile scalars: (128, CT) column per c-tile
            def col_tile(dram, name):
                tl = cp.tile([128, CT], f32, tag=name)
                src = dram.rearrange("(t p o) -> t p o", t=CT, o=1)
                for t in range(CT):
                    nc.sync.dma_start(out=tl[:, t:t + 1], in_=src[t])
                return tl

            gam_sb = col_tile(gam_d, "gam")
            bet_sb = col_tile(bet_d, "bet")
            b0_sb = col_tile(b_d[0], "b0")
            b1_sb = col_tile(b_d[1], "b1")

            # row-vector biases (1,C) rounded to f32r for K=1 matmul folds
            b2row = sp.tile([1, C], f32, tag="b2row", name="b2row")
            nc.sync.dma_start(out=b2row[:], in_=b_d[2].rearrange("(o c) -> o c", o=1))
            b2row_r = sp.tile([1, C], f32r, tag="b2rowr", name="b2rowr")
            nc.vector.tensor_copy(b2row_r[:], b2row[:])
            b3row = sp.tile([1, C], f32, tag="b3row", name="b3row")
            nc.sync.dma_start(out=b3row[:], in_=b_d[3].rearrange("(o c) -> o c", o=1))
            b3row_r = sp.tile([1, C], f32r, tag="b3rowr", name="b3rowr")
            nc.vector.tensor_copy(b3row_r[:], b3row[:])

            eps16 = sp.tile([G // CT, 1], f32, tag="eps16", name="eps16")
            nc.vector.memset(eps16[:], EPS)

            # ---------- GroupNorm ----------
            for t in range(CT):
                stats = sp.tile([128, 4, 6], f32, tag=f"stats{t}", name=f"stats{t}")
                for j in range(4):
                    nc.vector.bn_stats(out=stats[:, j, :],
                                       in_=xt[:, t, j * 512:(j + 1) * 512])
                mv = sp.tile([128, 2], f32, tag=f"mv{t}", name=f"mv{t}")
                nc.vector.bn_aggr(out=mv[:], in_=stats[:])

                s = sp.tile([128, 2], f32, tag=f"s{t}", name=f"s{t}")          # [mean, E[x^2]]
                nc.vector.tensor_copy(s[:, 0:1], mv[:, 0:1])
                nc.vector.tensor_mul(s[:, 1:2], mv[:, 0:1], mv[:, 0:1])
                nc.vector.tensor_add(s[:, 1:2], s[:, 1:2], mv[:, 1:2])

                gps = ps.tile([G // CT, 2], f32, tag="gn", name="gn")
                nc.tensor.matmul(gps[:], g8s_sb[:], s[:], start=True, stop=True)
                gsb = sp.tile([G // CT, 2], f32, tag=f"gsb{t}", name=f"gsb{t}")  # [mean_g, E2_g]
                nc.vector.tensor_copy(gsb[:], gps[:])

                gvar = sp.tile([G // CT, 1], f32, tag=f"gvar{t}", name=f"gvar{t}")
                nc.vector.tensor_mul(gvar[:], gsb[:, 0:1], gsb[:, 0:1])
                nc.vector.tensor_sub(gvar[:], gsb[:, 1:2], gvar[:])
                grs = sp.tile([G // CT, 2], f32, tag=f"grs{t}", name=f"grs{t}")  # [mean_g, rstd_g]
                nc.scalar.activation(out=grs[:, 1:2], in_=gvar[:], func=AF.Sqrt,
                                     bias=eps16[:], scale=1.0)
                nc.vector.reciprocal(grs[:, 1:2], grs[:, 1:2])
                nc.vector.tensor_copy(grs[:, 0:1], gsb[:, 0:1])

                bps = ps.tile([128, 2], f32, tag="gn", name="gn")
                nc.tensor.matmul(bps[:], g8t_sb[:], grs[:], start=True, stop=True)
                bc = sp.tile([128, 2], f32, tag=f"bc{t}", name=f"bc{t}")        # [mean_c, rstd_c]
                nc.vector.tensor_copy(bc[:], bps[:])

                A = sp.tile([128, 1], f32, tag=f"A{t}", name=f"A{t}")
                D = sp.tile([128, 1], f32, tag=f"D{t}", name=f"D{t}")
                nc.vector.tensor_mul(A[:], bc[:, 1:2], gam_sb[:, t:t + 1])
                nc.vector.tensor_mul(D[:], bc[:, 0:1], A[:])
                nc.vector.tensor_sub(D[:], bet_sb[:, t:t + 1], D[:])

                nc.scalar.activation(out=hn[:, t, :], in_=xt[:, t, :],
                                     func=AF.Identity, bias=D[:], scale=A[:])

            # ---------- q, k projections (channel-major) ----------
            for dst, wi, bias in ((qt, 0, b0_sb), (kt, 1, b1_sb)):
                for t in range(CT):          # output d-tile
                    for n in range(NCH):
                        mm = ps.tile([128, 512], f32, tag="mm", name="mm")
                        for k in range(CT):  # contraction c-tile
                            nc.tensor.matmul(
                                mm[:],
                                wr[wi][:, k, t * 128:(t + 1) * 128],
                                hn[:, k, n * 512:(n + 1) * 512],
                                start=(k == 0), stop=(k == CT - 1))
                        nc.vector.tensor_scalar_add(
                            dst[:, t, n * 512:(n + 1) * 512], mm[:], bias[:, t:t + 1])

            # ---------- v projection (transposed: vT[i, c]) ----------
            for ib in range(KB):
                mm = ps.tile([128, 512], f32, tag="mm", name="mm")
                for k in range(CT):
                    nc.tensor.matmul(
                        mm[:, 0:C],
                        hn[:, k, ib * 128:(ib + 1) * 128],
                        wr[2][:, k, :],
                        start=(k == 0), stop=False)
                nc.tensor.matmul(mm[:, 0:C], ones_r[0:1, :], b2row_r[:],
                                 start=False, stop=True)
                nc.vector.tensor_copy(vT[:, ib, :], mm[:, 0:C])

            # ---------- attention ----------
            for n in range(NCH):
                qs = slice(n * 512, (n + 1) * 512)
                pv = [ps.tile([128, 512], f32, tag=f"pv{t}", name=f"pv{t}") for t in range(CT)]
                rps = ps.tile([128, 512], f32, tag="rr", name="rr")
                for ib in range(KB):
                    st = ps.tile([128, 512], f32, tag="mm", name="mm")
                    for k in range(CT):
                        nc.tensor.matmul(
                            st[:],
                            kt[:, k, ib * 128:(ib + 1) * 128],
                            qt[:, k, qs],
                            start=(k == 0), stop=(k == CT - 1))
                    ex = ep.tile([128, 512], f32r, tag="expst", name="expst")
                    nc.scalar.activation(out=ex[:], in_=st[:], func=AF.Exp,
                                         scale=SCALE)
                    first, last = ib == 0, ib == KB - 1
                    for t in range(CT):
                        nc.tensor.matmul(pv[t][:], vT[:, ib, t * 128:(t + 1) * 128],
                                         ex[:], start=first, stop=last)
                    nc.tensor.matmul(rps[:], ones_r[:], ex[:], start=first, stop=last)

                rinv = fp_.tile([128, 512], f32, tag="rinv", name="rinv")
                scr = fp_.tile([128, 512], f32, tag="scr", name="scr")
                nc.vector.reciprocal_approx_accurate(out=rinv[:], in_=rps[:], scratch=scr[:])

                att = [ap_.tile([128, 512], f32r, tag=f"attn{t}", name=f"attn{t}") for t in range(CT)]
                for t in range(CT):
                    nc.vector.tensor_mul(att[t][:], pv[t][:], rinv[:])

                # output projection + bias + residual
                for t in range(CT):
                    mm = ps.tile([128, 512], f32, tag="mm", name="mm")
                    for k in range(CT):
                        nc.tensor.matmul(mm[:], wr[3][:, k, t * 128:(t + 1) * 128],
                                         att[k][:], start=(k == 0), stop=False)
                    nc.tensor.matmul(mm[:], b3row_r[0:1, t * 128:(t + 1) * 128],
                                     ones512_r[:], start=False, stop=True)
                    ob = fp_.tile([128, 512], f32, tag="outb", name="outb")
                    nc.vector.tensor_add(ob[:], mm[:], xt[:, t, qs])
                    nc.sync.dma_start(out=out_d[t * 128:(t + 1) * 128, qs], in_=ob[:])

    nc.compile()
    return nc


_NC_CACHE = {}


def _get_nc():
    if "nc" not in _NC_CACHE:
        _NC_CACHE["nc"] = _build()
    return _NC_CACHE["nc"]


def run(inputs, trace=False, **kw):
    nc = _get_nc()
    names = ["w0", "b0", "w1", "b1", "w2", "b2", "w3", "b3", "gn_gamma", "gn_beta"]
    shared = {k: np.ascontiguousarray(np.asarray(inputs[k], dtype=np.float32))
              for k in names}
    x = np.ascontiguousarray(np.asarray(inputs["x"], dtype=np.float32))
    in_maps = [dict(shared, x=x[b]) for b in range(B)]
    res = run_bass_kernel_spmd(nc, in_maps, core_ids=list(range(B)), trace=trace, **kw)
    out = np.stack([res.results[b]["out"] for b in range(B)], axis=0)
    return out, res


def kernel(**inputs) -> np.ndarray:
    out, _ = run(inputs)
    return out


def make_bench_runner(inputs):
    """Reusable jitted shard_map callable (no donation) + device-resident args,
    for amortized HW timing. Mirrors bass2jax.run_bass_via_pjrt."""
    import jax
    import concourse.mybir as _mybir
    from concourse import bass2jax as b2j
    from jax.experimental.shard_map import shard_map
    from jax.sharding import Mesh, PartitionSpec

    nc = _get_nc()
    b2j.install_neuronx_cc_hook()
    partition_name = nc.partition_id_tensor.name if nc.partition_id_tensor else None

    in_names, out_names, out_avals, zero_outs = [], [], [], []
    for alloc in nc.m.functions[0].allocations:
        if not isinstance(alloc, _mybir.MemoryLocationSet):
            continue
        name = alloc.memorylocations[0].name
        if alloc.kind == "ExternalInput":
            if name != partition_name:
                in_names.append(name)
        elif alloc.kind == "ExternalOutput":
            shape = tuple(alloc.tensor_shape)
            dtype = _mybir.dt.np(alloc.dtype)
            out_avals.append(jax.core.ShapedArray(shape, dtype))
            zero_outs.append(np.zeros(shape, dtype))
    n_params = len(in_names)
    all_names = list(in_names) + list(out_names := out_names or [
        a for a in ()
    ])
    # out names
    out_names = []
    for alloc in nc.m.functions[0].allocations:
        if isinstance(alloc, _mybir.MemoryLocationSet) and alloc.kind == "ExternalOutput":
            out_names.append(alloc.memorylocations[0].name)
    all_names = in_names + out_names
    if partition_name is not None:
        all_names.append(partition_name)

    def _body(*args):
        operands = list(args)
        if partition_name is not None:
            operands.append(b2j.partition_id_tensor())
        outs = b2j._bass_exec_p.bind(
            *operands,
            out_avals=tuple(out_avals),
            in_names=tuple(all_names),
            out_names=tuple(out_names),
            lowering_input_output_aliases=(),
            sim_require_finite=True,
            sim_require_nnan=True,
            nc=nc,
        )
        return tuple(outs)

    names = ["w0", "b0", "w1", "b1", "w2", "b2", "w3", "b3", "gn_gamma", "gn_beta"]
    shared = {k: np.ascontiguousarray(np.asarray(inputs[k], dtype=np.float32)) for k in names}
    x = np.ascontiguousarray(np.asarray(inputs["x"], dtype=np.float32))
    in_maps = [dict(shared, x=x[b]) for b in range(B)]

    devices = jax.devices()[:B]
    mesh = Mesh(np.asarray(devices), ("core",))
    nin = n_params + len(out_names)
    sharded = jax.jit(
        shard_map(_body, mesh=mesh,
                  in_specs=(PartitionSpec("core"),) * nin,
                  out_specs=(PartitionSpec("core"),) * len(out_names),
                  check_rep=False),
        keep_unused=True,
    )
    concat_in = [np.concatenate([in_maps[c][nm] for c in range(B)], axis=0)
                 for nm in in_names]
    concat_zeros = [np.zeros((B * z.shape[0], *z.shape[1:]), z.dtype) for z in zero_outs]
    args = [jax.device_put(a) for a in concat_in + concat_zeros]

    def call():
        return sharded(*args)

    return call, out_names, out_avals
